# revision 1
# baseline (speedup 1.0000x reference)
"""Trainium2 Bass kernel for GQA multi-head attention (TP-8 over heads).

Problem: hidden [1, 4096, 2048] fp32; wq [2048, 2048], wk/wv [2048, 512],
wo [2048, 2048]; 16 q-heads / 4 kv-heads, head_dim 128, interleaved RoPE,
causal softmax attention, o_proj.

Sharding: core c in 0..7 handles q-heads {2c, 2c+1} and kv-head c//2
(kv proj duplicated across core pairs). Each core produces a partial
o_proj output [4096, 2048] (fp16); the host sums the 8 partials in fp32.

All matmuls run in bf16 with fp32 PSUM accumulation (rel-err budget 2e-2).
Softmax runs without max-subtraction (logits*scale have |x| < ~6 here).
"""

import sys

sys.path.insert(0, "/opt/trn_rl_repo")

import math

import numpy as np

NUM_HEADS = 16
NUM_KV = 4
HD = 128
H = 2048
KVD = 512
ROPE_BASE = 10000.0
S_FULL = 4096
N_CORES = 8


def _rope_tables(S):
    inv = 1.0 / (ROPE_BASE ** (np.arange(0, HD, 2, dtype=np.float64) / HD))
    t = np.arange(S, dtype=np.float64)
    fr = t[:, None] * inv[None, :]  # [S, 64]
    cos = np.repeat(np.cos(fr), 2, axis=1)
    sin = np.repeat(np.sin(fr), 2, axis=1)
    sin2 = sin.copy()
    sin2[:, 0::2] *= -1.0  # even dims get -sin (r1 = x1*cos - x2*sin)
    return cos.astype(np.float32), sin2.astype(np.float32)


def _causal_masks():
    # mask[p, j, q] = 1 if q >= p + 128*j  (S^T tile layout: row = key pos,
    # col = query pos within the 512-wide chunk; j = k-tile slot in chunk)
    import ml_dtypes

    p = np.arange(128)[:, None, None]
    j = np.arange(4)[None, :, None]
    q = np.arange(512)[None, None, :]
    return (q >= p + 128 * j).astype(ml_dtypes.bfloat16)


def build(S=S_FULL):
    import ml_dtypes

    import concourse.bacc as bacc
    import concourse.mybir as mybir
    import concourse.tile as tile

    f32 = mybir.dt.float32
    bf16 = mybir.dt.bfloat16
    f16 = mybir.dt.float16
    AF = mybir.ActivationFunctionType

    NCH = S // 512  # 512-wide q chunks
    NT = S // 128  # 128-row s tiles
    KT = H // 128  # contraction tiles for projections
    scale = 1.0 / math.sqrt(HD)

    nc = bacc.Bacc("TRN2", target_bir_lowering=False, debug=False, num_devices=N_CORES)

    hid = nc.dram_tensor("hidden", [S, H], f32, kind="ExternalInput")
    wq = nc.dram_tensor("wq_s", [H, 2 * HD], f32, kind="ExternalInput")
    wk = nc.dram_tensor("wk_s", [H, HD], f32, kind="ExternalInput")
    wv = nc.dram_tensor("wv_s", [H, HD], f32, kind="ExternalInput")
    wo = nc.dram_tensor("wo_s", [2 * HD, H], f32, kind="ExternalInput")
    out = nc.dram_tensor("out_part", [S, H], f16, kind="ExternalOutput")

    cos_np, sin2_np = _rope_tables(S)
    cos_d = nc.inline_tensor(cos_np, name="cos_tab")
    sin_d = nc.inline_tensor(sin2_np, name="sin_tab")
    mask_d = nc.inline_tensor(_causal_masks(), name="causal_masks")
    ident_d = nc.inline_tensor(np.eye(128, dtype=ml_dtypes.bfloat16), name="ident")

    with tile.TileContext(nc) as tc:
        with tc.tile_pool(name="pers", bufs=1) as pers:
            qt0 = pers.tile([128, S], bf16, tag="qt0")
            qt1 = pers.tile([128, S], bf16, tag="qt1")
            kt = pers.tile([128, S], bf16, tag="kt")
            vnat = pers.tile([128, NT, HD], bf16, tag="vnat")
            mask_sb = pers.tile([128, 4, 512], bf16, tag="mask")
            ident_sb = pers.tile([128, 128], bf16, tag="ident")
            wcat = pers.tile([128, KT, 512], bf16, tag="wcat")
            wo_sb = pers.tile([128, 2, H], bf16, tag="wo")
            ones_c = pers.tile([128, 1], bf16, tag="ones_c")
            ones_r = pers.tile([1, 128], f32, tag="ones_r")

            nc.sync.dma_start(mask_sb[:], mask_d.ap())
            nc.sync.dma_start(ident_sb[:], ident_d.ap())
            nc.vector.memset(ones_c[:], 1.0)
            nc.vector.memset(ones_r[:], 1.0)

            # ---- weights: load fp32, cast to bf16 ----
            with tc.tile_pool(name="wstage", bufs=2) as wstage:
                stq = wstage.tile([128, KT, 2 * HD], f32, tag="w")
                nc.sync.dma_start(stq[:], wq.ap().rearrange("(T p) m -> p T m", p=128))
                nc.vector.tensor_copy(wcat[:, :, 0 : 2 * HD], stq[:])
                stk = wstage.tile([128, KT, HD], f32, tag="w")
                nc.sync.dma_start(stk[:], wk.ap().rearrange("(T p) m -> p T m", p=128))
                nc.vector.tensor_copy(wcat[:, :, 2 * HD : 3 * HD], stk[:])
                stv = wstage.tile([128, KT, HD], f32, tag="w")
                nc.sync.dma_start(stv[:], wv.ap().rearrange("(T p) m -> p T m", p=128))
                nc.vector.tensor_copy(wcat[:, :, 3 * HD : 4 * HD], stv[:])
                sto = wstage.tile([128, 2, H], f32, tag="w")
                nc.sync.dma_start(sto[:], wo.ap().rearrange("(T p) m -> p T m", p=128))
                nc.vector.tensor_copy(wo_sb[:], sto[:])

            # ---- phase 1: cast + transpose hidden, QKV proj, RoPE, Qt/Kt ----
            with (
                tc.tile_pool(name="dram", bufs=1, space="DRAM") as dramp,
                tc.tile_pool(name="hidst", bufs=2) as hidst,
                tc.tile_pool(name="hbf", bufs=2) as hbfp,
                tc.tile_pool(name="hT", bufs=2 * KT) as hTp,
                tc.tile_pool(name="trig", bufs=2) as trigp,
                tc.tile_pool(name="rope", bufs=8) as ropep,
                tc.tile_pool(name="qkvnat", bufs=3) as natp,
                tc.tile_pool(name="psproj", bufs=2, space="PSUM") as ps_proj,
                tc.tile_pool(name="pstr", bufs=2, space="PSUM") as ps_tr,
            ):
                hid_bf = dramp.tile([S, H], bf16)
                for i in range(NCH):
                    for t in range(4):
                        g = 4 * i + t
                        st = hidst.tile([128, H], f32, tag="hidst")
                        nc.sync.dma_start(st[:], hid.ap()[128 * g : 128 * (g + 1), :])
                        hb = hbfp.tile([128, H], bf16, tag="hbf")
                        nc.vector.tensor_copy(hb[:], st[:])
                        nc.sync.dma_start(hid_bf[128 * g : 128 * (g + 1), :], hb[:])

                    cos_ch = trigp.tile([128, 4, HD], f32, tag="cos")
                    nc.sync.dma_start(
                        cos_ch[:],
                        cos_d.ap()[512 * i : 512 * (i + 1), :].rearrange(
                            "(q p) d -> p q d", p=128
                        ),
                    )
                    sin_ch = trigp.tile([128, 4, HD], f32, tag="sin")
                    nc.sync.dma_start(
                        sin_ch[:],
                        sin_d.ap()[512 * i : 512 * (i + 1), :].rearrange(
                            "(q p) d -> p q d", p=128
                        ),
                    )

                    hts = []
                    for k in range(KT):
                        ht = hTp.tile([128, 512], bf16, tag="hT")
                        nc.sync.dma_start_transpose(
                            ht[:],
                            hid_bf[512 * i : 512 * (i + 1), 128 * k : 128 * (k + 1)],
                        )
                        hts.append(ht)

                    for t in range(4):
                        g = 4 * i + t
                        pq = ps_proj.tile([128, 512], f32, tag="proj")
                        for k in range(KT):
                            nc.tensor.matmul(
                                pq[:],
                                hts[k][:, 128 * t : 128 * (t + 1)],
                                wcat[:, k, :],
                                start=(k == 0),
                                stop=(k == KT - 1),
                            )
                        # V: plain cast drain to natural layout
                        nc.scalar.copy(vnat[:, g, :], pq[:, 384:512])
                        # RoPE on q0 | q1 | k then transpose into Qt/Kt
                        nat = natp.tile([128, 384], bf16, tag="nat")
                        for j in range(3):
                            xs = pq[:, 128 * j : 128 * (j + 1)]
                            xv = xs.rearrange("p (i two) -> p two i", two=2)
                            xsw = ropep.tile([128, 128], f32, tag="xsw")
                            xwv = xsw[:].rearrange("p (i two) -> p two i", two=2)
                            nc.vector.tensor_copy(xwv[:, 0, :], xv[:, 1, :])
                            nc.vector.tensor_copy(xwv[:, 1, :], xv[:, 0, :])
                            t1 = ropep.tile([128, 128], f32, tag="t1")
                            nc.vector.tensor_mul(t1[:], xs, cos_ch[:, t, :])
                            t2 = ropep.tile([128, 128], f32, tag="t2")
                            nc.vector.tensor_mul(t2[:], xsw[:], sin_ch[:, t, :])
                            nc.vector.tensor_add(
                                nat[:, 128 * j : 128 * (j + 1)], t1[:], t2[:]
                            )
                        for j, dst in enumerate([qt0, qt1, kt]):
                            tp = ps_tr.tile([128, 128], bf16, tag="tr")
                            nc.tensor.transpose(
                                tp[:], nat[:, 128 * j : 128 * (j + 1)], ident_sb[:]
                            )
                            nc.vector.tensor_copy(
                                dst[:, 128 * g : 128 * (g + 1)], tp[:]
                            )

            # ---- phase 2: attention + o_proj ----
            with (
                tc.tile_pool(name="psatt", bufs=4, space="PSUM") as ps_att,
                tc.tile_pool(name="psacc", bufs=2, space="PSUM") as ps_acc,
                tc.tile_pool(name="pso", bufs=2, space="PSUM") as ps_o,
                tc.tile_pool(name="pt", bufs=4) as ptp,
                tc.tile_pool(name="atn", bufs=4) as atnp,
                tc.tile_pool(name="rcp", bufs=4) as rcpp,
                tc.tile_pool(name="ost", bufs=2) as ostp,
            ):
                for i in range(NCH):
                    atn_h = []
                    for h, QT in enumerate([qt0, qt1]):
                        acc = ps_acc.tile([128, 512], f32, tag="acc")
                        den = ps_att.tile([1, 512], f32, tag="att")
                        nk = 4 * (i + 1)
                        for kk in range(nk):
                            sc = ps_att.tile([128, 512], f32, tag="att")
                            nc.tensor.matmul(
                                sc[:],
                                kt[:, 128 * kk : 128 * (kk + 1)],
                                QT[:, 512 * i : 512 * (i + 1)],
                                start=True,
                                stop=True,
                            )
                            pt = ptp.tile([128, 512], bf16, tag="pt")
                            nc.scalar.activation(pt[:], sc[:], AF.Exp, scale=scale)
                            if kk >= 4 * i:
                                nc.vector.tensor_mul(
                                    pt[:], pt[:], mask_sb[:, kk - 4 * i, :]
                                )
                            nc.tensor.matmul(
                                acc[:],
                                vnat[:, kk, :],
                                pt[:],
                                start=(kk == 0),
                                stop=(kk == nk - 1),
                            )
                            nc.tensor.matmul(
                                den[:],
                                ones_c[:],
                                pt[:],
                                start=(kk == 0),
                                stop=(kk == nk - 1),
                                skip_group_check=True,
                            )
                        rc = rcpp.tile([1, 512], f32, tag="rc")
                        nc.vector.reciprocal(rc[:], den[:])
                        rb = ps_att.tile([128, 512], f32, tag="att")
                        nc.tensor.matmul(rb[:], ones_r[:], rc[:], start=True, stop=True)
                        rbs = rcpp.tile([128, 512], f32, tag="rbs")
                        nc.vector.tensor_copy(rbs[:], rb[:])
                        an = atnp.tile([128, 512], bf16, tag="atn")
                        nc.vector.tensor_mul(an[:], acc[:], rbs[:])
                        atn_h.append(an)

                    for t in range(4):
                        g = 4 * i + t
                        ost = ostp.tile([128, H], f16, tag="ost")
                        for nn in range(0, 4, 2):
                            poa = ps_o.tile([128, 512], f32, tag="o")
                            pob = ps_o.tile([128, 512], f32, tag="o")
                            for h in range(2):
                                lhs = atn_h[h][:, 128 * t : 128 * (t + 1)]
                                nc.tensor.matmul(
                                    poa[:],
                                    lhs,
                                    wo_sb[:, h, 512 * nn : 512 * (nn + 1)],
                                    start=(h == 0),
                                    stop=(h == 1),
                                    skip_group_check=True,
                                )
                                nc.tensor.matmul(
                                    pob[:],
                                    lhs,
                                    wo_sb[:, h, 512 * (nn + 1) : 512 * (nn + 2)],
                                    start=(h == 0),
                                    stop=(h == 1),
                                    skip_group_check=True,
                                )
                            nc.scalar.copy(ost[:, 512 * nn : 512 * (nn + 1)], poa[:])
                            nc.scalar.copy(
                                ost[:, 512 * (nn + 1) : 512 * (nn + 2)], pob[:]
                            )
                        nc.sync.dma_start(
                            out.ap()[128 * g : 128 * (g + 1), :], ost[:]
                        )

    nc.compile()
    return nc


_CACHE = {}


def _get_program(S=S_FULL):
    if S not in _CACHE:
        _CACHE[S] = build(S)
    return _CACHE[S]


def shard_inputs(hidden_states, wq, wk, wv, wo):
    S = hidden_states.shape[1]
    hid2d = np.ascontiguousarray(hidden_states.reshape(S, H).astype(np.float32))
    in_maps = []
    for c in range(N_CORES):
        g = c // 2
        in_maps.append(
            {
                "hidden": hid2d,
                "wq_s": np.ascontiguousarray(wq[:, 256 * c : 256 * (c + 1)]),
                "wk_s": np.ascontiguousarray(wk[:, 128 * g : 128 * (g + 1)]),
                "wv_s": np.ascontiguousarray(wv[:, 128 * g : 128 * (g + 1)]),
                "wo_s": np.ascontiguousarray(wo[256 * c : 256 * (c + 1), :]),
            }
        )
    return in_maps


def kernel(hidden_states, wq, wk, wv, wo, _trace=False):
    from concourse import bass_utils

    B, S, _ = hidden_states.shape
    nc = _get_program(S)
    in_maps = shard_inputs(hidden_states, wq, wk, wv, wo)
    res = bass_utils.run_bass_kernel_spmd(
        nc, in_maps, core_ids=list(range(N_CORES)), trace=_trace
    )
    acc = np.zeros((S, H), dtype=np.float32)
    for c in range(N_CORES):
        acc += res.results[c]["out_part"].astype(np.float32)
    out = acc.reshape(B, S, H)
    if _trace:
        return out, res
    return out


# revision 5
# speedup vs baseline: 1.5336x; 1.5336x over previous
"""Trainium2 Bass kernel for GQA multi-head attention (TP-8 over heads).

Problem: hidden [1, 4096, 2048] fp32; wq [2048, 2048], wk/wv [2048, 512],
wo [2048, 2048]; 16 q-heads / 4 kv-heads, head_dim 128, interleaved RoPE,
causal softmax attention, o_proj.

Sharding: core c in 0..7 handles q-heads {2c, 2c+1} and kv-head c//2
(kv proj duplicated across core pairs). Each core produces a partial
o_proj output [4096, 2048] (fp16); the host sums the 8 partials in fp32.

v3 structure (engine balance measured on HW):
- hidden + weights cast to bf16 on the host; hidden^T obtained on-device
  via DMA-transpose loads straight from the bf16 input (no roundtrip).
- QKV projection form A: stationary hT[k, s128], moving [wq0|wq1|wk|wv],
  natural-layout output.
- RoPE in natural layout, all-bf16, pair-swap via strided APs (DVE).
- Q/K transposed to [d, s] via PE-transpose + DVE drain; V kept natural.
- Attention in scores-transposed layout [k, q]: softmax needs no
  max-subtraction (logits are small), exp on ACT, causal mask via gpsimd
  affine_select, denominator accumulated in fp16 on DVE, normalization
  via ones-matmul reduce + gpsimd partition_broadcast of the reciprocal.
- o_proj: stationary attnT slices, PSUM drains split DVE/ACT.
"""

import sys

sys.path.insert(0, "/opt/trn_rl_repo")

import math

import numpy as np

NUM_HEADS = 16
NUM_KV = 4
HD = 128
H = 2048
KVD = 512
ROPE_BASE = 10000.0
S_FULL = 4096
N_CORES = 8


def _rope_tables(S):
    inv = 1.0 / (ROPE_BASE ** (np.arange(0, HD, 2, dtype=np.float64) / HD))
    t = np.arange(S, dtype=np.float64)
    fr = t[:, None] * inv[None, :]  # [S, 64]
    cos = np.repeat(np.cos(fr), 2, axis=1)
    sin = np.repeat(np.sin(fr), 2, axis=1)
    sin2 = sin.copy()
    sin2[:, 0::2] *= -1.0  # even dims get -sin (r1 = x1*cos - x2*sin)
    return cos, sin2


def _rope_tables_3x(S):
    import ml_dtypes

    cos, sin2 = _rope_tables(S)
    cos3 = np.tile(cos, (1, 3)).astype(ml_dtypes.bfloat16)
    sin3 = np.tile(sin2, (1, 3)).astype(ml_dtypes.bfloat16)
    return cos3, sin3


def build(S=S_FULL):
    import ml_dtypes

    import concourse.bacc as bacc
    import concourse.mybir as mybir
    import concourse.tile as tile

    f32 = mybir.dt.float32
    bf16 = mybir.dt.bfloat16
    f16 = mybir.dt.float16
    AF = mybir.ActivationFunctionType
    ALU = mybir.AluOpType

    NCH = S // 512
    NT = S // 128
    KT = H // 128
    scale = 1.0 / math.sqrt(HD)

    nc = bacc.Bacc("TRN2", target_bir_lowering=False, debug=False, num_devices=N_CORES)

    hid = nc.dram_tensor("hidden", [S, H], bf16, kind="ExternalInput")
    wq = nc.dram_tensor("wq_s", [H, 2 * HD], bf16, kind="ExternalInput")
    wk = nc.dram_tensor("wk_s", [H, HD], bf16, kind="ExternalInput")
    wv = nc.dram_tensor("wv_s", [H, HD], bf16, kind="ExternalInput")
    wo = nc.dram_tensor("wo_s", [2 * HD, H], bf16, kind="ExternalInput")
    out = nc.dram_tensor("out_part", [S, H], f16, kind="ExternalOutput")

    cos3_np, sin3_np = _rope_tables_3x(S)
    cos_d = nc.inline_tensor(cos3_np, name="cos_tab")
    sin_d = nc.inline_tensor(sin3_np, name="sin_tab")
    ident_d = nc.inline_tensor(np.eye(128, dtype=ml_dtypes.bfloat16), name="ident")

    with tile.TileContext(nc) as tc:
        with tc.tile_pool(name="pers", bufs=1) as pers:
            qt0 = pers.tile([128, S], bf16, tag="qt0")
            qt1 = pers.tile([128, S], bf16, tag="qt1")
            kt = pers.tile([128, S], bf16, tag="kt")
            vnat = pers.tile([128, NT, HD], bf16, tag="vnat")
            wcat = pers.tile([128, KT, 512], bf16, tag="wcat")
            wo_sb = pers.tile([128, 2, H], bf16, tag="wo")
            ones_h = pers.tile([128, 1], f16, tag="ones_h")
            ident_sb = pers.tile([128, 128], bf16, tag="ident")
            nc.vector.memset(ones_h[:], 1.0)
            nc.sync.dma_start(ident_sb[:], ident_d.ap())

            nc.sync.dma_start(
                wcat[:, :, 0 : 2 * HD],
                wq.ap().rearrange("(T p) m -> p T m", p=128),
            )
            nc.sync.dma_start(
                wcat[:, :, 2 * HD : 3 * HD],
                wk.ap().rearrange("(T p) m -> p T m", p=128),
            )
            nc.sync.dma_start(
                wcat[:, :, 3 * HD : 4 * HD],
                wv.ap().rearrange("(T p) m -> p T m", p=128),
            )
            nc.sync.dma_start(
                wo_sb[:], wo.ap().rearrange("(T p) m -> p T m", p=128)
            )

            # ---- phase 1: transpose-load hidden, QKV proj, RoPE, Qt/Kt ----
            with (
                tc.tile_pool(name="hT", bufs=2 * KT) as hTp,
                tc.tile_pool(name="trig", bufs=2) as trigp,
                tc.tile_pool(name="rope", bufs=6) as ropep,
                tc.tile_pool(name="psproj", bufs=2, space="PSUM") as ps_proj,
                tc.tile_pool(name="pstr", bufs=3, space="PSUM") as ps_tr,
            ):
                for i in range(NCH):
                    cos_ch = trigp.tile([128, 4, 3 * HD], bf16, tag="cos")
                    nc.sync.dma_start(
                        cos_ch[:],
                        cos_d.ap()[512 * i : 512 * (i + 1), :].rearrange(
                            "(q p) d -> p q d", p=128
                        ),
                    )
                    sin_ch = trigp.tile([128, 4, 3 * HD], bf16, tag="sin")
                    nc.sync.dma_start(
                        sin_ch[:],
                        sin_d.ap()[512 * i : 512 * (i + 1), :].rearrange(
                            "(q p) d -> p q d", p=128
                        ),
                    )

                    hts = [
                        hTp.tile([128, 512], bf16, tag="hT", name=f"hT_{i}_{k}")
                        for k in range(KT)
                    ]
                    for k in range(KT):
                        nc.sync.dma_start_transpose(
                            hts[k][:],
                            hid.ap()[
                                512 * i : 512 * (i + 1), 128 * k : 128 * (k + 1)
                            ],
                        )

                    for t in range(4):
                        g = 4 * i + t
                        pq = ps_proj.tile([128, 512], f32, tag="proj")
                        for k in range(KT):
                            nc.tensor.matmul(
                                pq[:],
                                hts[k][:, 128 * t : 128 * (t + 1)],
                                wcat[:, k, :],
                                start=(k == 0),
                                stop=(k == KT - 1),
                            )
                        # drain whole tile to natural bf16 (includes V)
                        nat = ropep.tile([128, 512], bf16, tag="nat")
                        nc.scalar.copy(nat[:], pq[:])
                        nc.vector.tensor_copy(vnat[:, g, :], nat[:, 384:512])
                        # RoPE on q0|q1|k: rot = nat*cos + natswap*sin2
                        natp = nat[:, 0:384].rearrange("p (i two) -> p two i", two=2)
                        xsw = ropep.tile([128, 384], bf16, tag="xsw")
                        xwv = xsw[:].rearrange("p (i two) -> p two i", two=2)
                        nc.vector.tensor_copy(xwv[:, 0, :], natp[:, 1, :])
                        nc.vector.tensor_copy(xwv[:, 1, :], natp[:, 0, :])
                        t1 = ropep.tile([128, 384], bf16, tag="t1")
                        nc.vector.tensor_mul(t1[:], nat[:, 0:384], cos_ch[:, t, :])
                        rot = ropep.tile([128, 384], bf16, tag="rot")
                        nc.vector.tensor_mul(xsw[:], xsw[:], sin_ch[:, t, :])
                        nc.vector.tensor_add(rot[:], t1[:], xsw[:])
                        for j, dst in enumerate([qt0, qt1, kt]):
                            tp = ps_tr.tile([128, 128], bf16, tag="tr")
                            nc.tensor.transpose(
                                tp[:], rot[:, 128 * j : 128 * (j + 1)], ident_sb[:]
                            )
                            nc.vector.tensor_copy(
                                dst[:, 128 * g : 128 * (g + 1)], tp[:]
                            )

            # ---- phase 2: attention + o_proj ----
            with (
                tc.tile_pool(name="psatt", bufs=3, space="PSUM") as ps_att,
                tc.tile_pool(name="psacc", bufs=2, space="PSUM") as ps_acc,
                tc.tile_pool(name="psden", bufs=1, space="PSUM") as ps_den,
                tc.tile_pool(name="pso", bufs=2, space="PSUM") as ps_o,
                tc.tile_pool(name="pt", bufs=4) as ptp,
                tc.tile_pool(name="den", bufs=2) as denp,
                tc.tile_pool(name="atn", bufs=4) as atnp,
                tc.tile_pool(name="rcp", bufs=2) as rcpp,
                tc.tile_pool(name="ost", bufs=2) as ostp,
            ):
                drain_flip = 0
                for i in range(NCH):
                    atn_h = []
                    for h, QT in enumerate([qt0, qt1]):
                        acc = ps_acc.tile([128, 512], f32, tag="acc")
                        den = denp.tile([128, 512], f16, tag="den")
                        nk = 4 * (i + 1)
                        for kk in range(nk):
                            sc = ps_att.tile([128, 512], f32, tag="att")
                            nc.tensor.matmul(
                                sc[:],
                                kt[:, 128 * kk : 128 * (kk + 1)],
                                QT[:, 512 * i : 512 * (i + 1)],
                                start=True,
                                stop=True,
                            )
                            pt = ptp.tile([128, 512], bf16, tag="pt")
                            nc.scalar.activation(pt[:], sc[:], AF.Exp, scale=scale)
                            if kk >= 4 * i:
                                # causal: keep pt[p, q] iff q - p - 128*(kk-4i) >= 0
                                nc.gpsimd.affine_select(
                                    pt[:],
                                    pt[:],
                                    [[1, 512]],
                                    ALU.is_ge,
                                    0.0,
                                    base=-128 * (kk - 4 * i),
                                    channel_multiplier=-1,
                                )
                            nc.tensor.matmul(
                                acc[:],
                                vnat[:, kk, :],
                                pt[:],
                                start=(kk == 0),
                                stop=(kk == nk - 1),
                            )
                            if kk == 0:
                                nc.vector.tensor_copy(den[:], pt[:])
                            else:
                                nc.vector.tensor_add(den[:], den[:], pt[:])
                        dps = ps_den.tile([1, 512], f32, tag="den")
                        nc.tensor.matmul(
                            dps[:], ones_h[:], den[:], start=True, stop=True
                        )
                        rc = rcpp.tile([1, 512], f32, tag="rc")
                        nc.vector.reciprocal(rc[:], dps[:])
                        rb = rcpp.tile([128, 512], f32, tag="rb")
                        nc.gpsimd.partition_broadcast(rb[:], rc[:])
                        an = atnp.tile([128, 512], bf16, tag="atn")
                        nc.vector.tensor_mul(an[:], acc[:], rb[:])
                        atn_h.append(an)

                    for t in range(4):
                        g = 4 * i + t
                        ost = ostp.tile([128, H], f16, tag="ost")
                        for nn in range(0, 4, 2):
                            poa = ps_o.tile([128, 512], f32, tag="o")
                            pob = ps_o.tile([128, 512], f32, tag="o")
                            for h in range(2):
                                lhs = atn_h[h][:, 128 * t : 128 * (t + 1)]
                                nc.tensor.matmul(
                                    poa[:],
                                    lhs,
                                    wo_sb[:, h, 512 * nn : 512 * (nn + 1)],
                                    start=(h == 0),
                                    stop=(h == 1),
                                    skip_group_check=True,
                                )
                                nc.tensor.matmul(
                                    pob[:],
                                    lhs,
                                    wo_sb[:, h, 512 * (nn + 1) : 512 * (nn + 2)],
                                    start=(h == 0),
                                    stop=(h == 1),
                                    skip_group_check=True,
                                )
                            # alternate drains between DVE and ACT to balance load
                            for ptile, col in ((poa, nn), (pob, nn + 1)):
                                if drain_flip % 4 != 0:
                                    nc.vector.tensor_copy(
                                        ost[:, 512 * col : 512 * (col + 1)], ptile[:]
                                    )
                                else:
                                    nc.scalar.copy(
                                        ost[:, 512 * col : 512 * (col + 1)], ptile[:]
                                    )
                                drain_flip += 1
                        nc.sync.dma_start(
                            out.ap()[128 * g : 128 * (g + 1), :], ost[:]
                        )

    nc.compile()
    return nc


_CACHE = {}


def _get_program(S=S_FULL):
    if S not in _CACHE:
        _CACHE[S] = build(S)
    return _CACHE[S]


def shard_inputs(hidden_states, wq, wk, wv, wo):
    import ml_dtypes

    bf = ml_dtypes.bfloat16
    S = hidden_states.shape[1]
    hid2d = np.ascontiguousarray(hidden_states.reshape(S, H).astype(bf))
    wqb = wq.astype(bf)
    wkb = wk.astype(bf)
    wvb = wv.astype(bf)
    wob = wo.astype(bf)
    in_maps = []
    for c in range(N_CORES):
        g = c // 2
        in_maps.append(
            {
                "hidden": hid2d,
                "wq_s": np.ascontiguousarray(wqb[:, 256 * c : 256 * (c + 1)]),
                "wk_s": np.ascontiguousarray(wkb[:, 128 * g : 128 * (g + 1)]),
                "wv_s": np.ascontiguousarray(wvb[:, 128 * g : 128 * (g + 1)]),
                "wo_s": np.ascontiguousarray(wob[256 * c : 256 * (c + 1), :]),
            }
        )
    return in_maps


def kernel(hidden_states, wq, wk, wv, wo, _trace=False):
    from concourse import bass_utils

    B, S, _ = hidden_states.shape
    nc = _get_program(S)
    in_maps = shard_inputs(hidden_states, wq, wk, wv, wo)
    res = bass_utils.run_bass_kernel_spmd(
        nc, in_maps, core_ids=list(range(N_CORES)), trace=_trace
    )
    acc = np.zeros((S, H), dtype=np.float32)
    for c in range(N_CORES):
        acc += res.results[c]["out_part"].astype(np.float32)
    out = acc.reshape(B, S, H)
    if _trace:
        return out, res
    return out


# revision 11
# speedup vs baseline: 1.5784x; 1.0292x over previous
"""Trainium2 Bass kernel for GQA multi-head attention (TP-8 over heads).

Problem: hidden [1, 4096, 2048] fp32; wq [2048, 2048], wk/wv [2048, 512],
wo [2048, 2048]; 16 q-heads / 4 kv-heads, head_dim 128, interleaved RoPE,
causal softmax attention, o_proj.

Sharding: core c in 0..7 handles q-heads {2c, 2c+1} and kv-head c//2
(kv proj duplicated across core pairs). Each core produces a partial
o_proj output [4096, 2048] (fp16); the host sums the 8 partials in fp32.

v3 structure (engine balance measured on HW):
- hidden + weights cast to bf16 on the host; hidden^T obtained on-device
  via DMA-transpose loads straight from the bf16 input (no roundtrip).
- QKV projection form A: stationary hT[k, s128], moving [wq0|wq1|wk|wv],
  natural-layout output.
- RoPE in natural layout, all-bf16, pair-swap via strided APs (DVE).
- Q/K transposed to [d, s] via PE-transpose + DVE drain; V kept natural.
- Attention in scores-transposed layout [k, q]: softmax needs no
  max-subtraction (logits are small), exp on ACT, causal mask via gpsimd
  affine_select, denominator accumulated in fp16 on DVE, normalization
  via ones-matmul reduce + gpsimd partition_broadcast of the reciprocal.
- o_proj: stationary attnT slices, PSUM drains split DVE/ACT.
"""

import sys

sys.path.insert(0, "/opt/trn_rl_repo")

import math

import numpy as np

NUM_HEADS = 16
NUM_KV = 4
HD = 128
H = 2048
KVD = 512
ROPE_BASE = 10000.0
S_FULL = 4096
N_CORES = 8


def _rope_tables(S):
    inv = 1.0 / (ROPE_BASE ** (np.arange(0, HD, 2, dtype=np.float64) / HD))
    t = np.arange(S, dtype=np.float64)
    fr = t[:, None] * inv[None, :]  # [S, 64]
    cos = np.repeat(np.cos(fr), 2, axis=1)
    sin = np.repeat(np.sin(fr), 2, axis=1)
    sin2 = sin.copy()
    sin2[:, 0::2] *= -1.0  # even dims get -sin (r1 = x1*cos - x2*sin)
    return cos, sin2


def _rope_tables_3x(S):
    import ml_dtypes

    cos, sin2 = _rope_tables(S)
    cos3 = np.tile(cos, (1, 3)).astype(ml_dtypes.bfloat16)
    sin3 = np.tile(sin2, (1, 3)).astype(ml_dtypes.bfloat16)
    return cos3, sin3


def build(S=S_FULL):
    import ml_dtypes

    import concourse.bacc as bacc
    import concourse.mybir as mybir
    import concourse.tile as tile

    f32 = mybir.dt.float32
    bf16 = mybir.dt.bfloat16
    f16 = mybir.dt.float16
    AF = mybir.ActivationFunctionType
    ALU = mybir.AluOpType

    NCH = S // 512
    NT = S // 128
    KT = H // 128
    scale = 1.0 / math.sqrt(HD)

    nc = bacc.Bacc("TRN2", target_bir_lowering=False, debug=False, num_devices=N_CORES)

    hid = nc.dram_tensor("hidden", [S, H], bf16, kind="ExternalInput")
    wq = nc.dram_tensor("wq_s", [H, 2 * HD], bf16, kind="ExternalInput")
    wk = nc.dram_tensor("wk_s", [H, HD], bf16, kind="ExternalInput")
    wv = nc.dram_tensor("wv_s", [H, HD], bf16, kind="ExternalInput")
    wo = nc.dram_tensor("wo_s", [2 * HD, H], bf16, kind="ExternalInput")
    out = nc.dram_tensor("out_part", [S, H], f16, kind="ExternalOutput")

    cos3_np, sin3_np = _rope_tables_3x(S)
    cos_d = nc.inline_tensor(cos3_np, name="cos_tab")
    sin_d = nc.inline_tensor(sin3_np, name="sin_tab")
    ident_d = nc.inline_tensor(np.eye(128, dtype=ml_dtypes.bfloat16), name="ident")

    with tile.TileContext(nc) as tc:
        with tc.tile_pool(name="pers", bufs=1) as pers:
            qt0 = pers.tile([128, S], bf16, tag="qt0")
            qt1 = pers.tile([128, S], bf16, tag="qt1")
            kt = pers.tile([128, S], bf16, tag="kt")
            vnat = pers.tile([128, NT, HD], bf16, tag="vnat")
            wcat = pers.tile([128, KT, 512], bf16, tag="wcat")
            wo_sb = pers.tile([128, 2, H], bf16, tag="wo")
            ones_m = pers.tile([128, 128], f16, tag="ones_m")
            ident_sb = pers.tile([128, 128], bf16, tag="ident")
            nc.vector.memset(ones_m[:], 1.0)
            nc.gpsimd.dma_start(ident_sb[:], ident_d.ap())

            nc.gpsimd.dma_start(
                wcat[:, :, 0 : 2 * HD],
                wq.ap().rearrange("(T p) m -> p T m", p=128),
            )
            nc.gpsimd.dma_start(
                wcat[:, :, 2 * HD : 3 * HD],
                wk.ap().rearrange("(T p) m -> p T m", p=128),
            )
            nc.gpsimd.dma_start(
                wcat[:, :, 3 * HD : 4 * HD],
                wv.ap().rearrange("(T p) m -> p T m", p=128),
            )
            nc.gpsimd.dma_start(
                wo_sb[:], wo.ap().rearrange("(T p) m -> p T m", p=128)
            )

            # ---- phase 1: transpose-load hidden, QKV proj, RoPE, Qt/Kt ----
            MC = 1024 if S % 1024 == 0 else 512  # macro-chunk rows per tr-load
            NTM = MC // 128
            with (
                tc.tile_pool(name="hT", bufs=int(1.5 * KT)) as hTp,
                tc.tile_pool(name="trig", bufs=2) as trigp,
                tc.tile_pool(name="rope", bufs=6) as ropep,
                tc.tile_pool(name="psproj", bufs=2, space="PSUM") as ps_proj,
                tc.tile_pool(name="pstr", bufs=3, space="PSUM") as ps_tr,
            ):
                for i in range(S // MC):
                    cos_ch = trigp.tile([128, NTM, 3 * HD], bf16, tag="cos")
                    nc.gpsimd.dma_start(
                        cos_ch[:],
                        cos_d.ap()[MC * i : MC * (i + 1), :].rearrange(
                            "(q p) d -> p q d", p=128
                        ),
                    )
                    sin_ch = trigp.tile([128, NTM, 3 * HD], bf16, tag="sin")
                    nc.gpsimd.dma_start(
                        sin_ch[:],
                        sin_d.ap()[MC * i : MC * (i + 1), :].rearrange(
                            "(q p) d -> p q d", p=128
                        ),
                    )

                    hts = [
                        hTp.tile([128, MC], bf16, tag="hT", name=f"hT_{i}_{k}")
                        for k in range(KT)
                    ]
                    for k in range(KT):
                        # all transpose-DMAs on ONE queue: the xbar is shared
                        # hardware; concurrent transposes on two queues corrupt
                        nc.sync.dma_start_transpose(
                            hts[k][:],
                            hid.ap()[MC * i : MC * (i + 1), 128 * k : 128 * (k + 1)],
                        )

                    for t in range(NTM):
                        g = NTM * i + t
                        pq = ps_proj.tile([128, 512], f32, tag="proj")
                        for k in range(KT):
                            nc.tensor.matmul(
                                pq[:],
                                hts[k][:, 128 * t : 128 * (t + 1)],
                                wcat[:, k, :],
                                start=(k == 0),
                                stop=(k == KT - 1),
                            )
                        # drain whole tile to natural bf16 (includes V)
                        nat = ropep.tile([128, 512], bf16, tag="nat")
                        nc.scalar.copy(nat[:], pq[:])
                        nc.vector.tensor_copy(vnat[:, g, :], nat[:, 384:512])
                        # RoPE on q0|q1|k: rot = nat*cos + natswap*sin2
                        natp = nat[:, 0:384].rearrange("p (i two) -> p two i", two=2)
                        xsw = ropep.tile([128, 384], bf16, tag="xsw")
                        xwv = xsw[:].rearrange("p (i two) -> p two i", two=2)
                        nc.vector.tensor_copy(xwv[:, 0, :], natp[:, 1, :])
                        nc.vector.tensor_copy(xwv[:, 1, :], natp[:, 0, :])
                        t1 = ropep.tile([128, 384], bf16, tag="t1")
                        nc.vector.tensor_mul(t1[:], nat[:, 0:384], cos_ch[:, t, :])
                        rot = ropep.tile([128, 384], bf16, tag="rot")
                        nc.vector.tensor_mul(xsw[:], xsw[:], sin_ch[:, t, :])
                        nc.vector.tensor_add(rot[:], t1[:], xsw[:])
                        for j, dst in enumerate([qt0, qt1, kt]):
                            tp = ps_tr.tile([128, 128], bf16, tag="tr")
                            nc.tensor.transpose(
                                tp[:], rot[:, 128 * j : 128 * (j + 1)], ident_sb[:]
                            )
                            nc.vector.tensor_copy(
                                dst[:, 128 * g : 128 * (g + 1)], tp[:]
                            )

            # ---- phase 2: attention + o_proj (both heads per k-pass) ----
            with (
                tc.tile_pool(name="psatt", bufs=2, space="PSUM") as ps_att,
                tc.tile_pool(name="psacc", bufs=2, space="PSUM") as ps_acc,
                tc.tile_pool(name="pso", bufs=2, space="PSUM") as ps_o,
                tc.tile_pool(name="pt", bufs=4) as ptp,
                tc.tile_pool(name="den", bufs=2) as denp,
                tc.tile_pool(name="atn", bufs=4) as atnp,
                tc.tile_pool(name="rcp", bufs=2) as rcpp,
                tc.tile_pool(name="ost", bufs=2) as ostp,
            ):
                drain_flip = 0
                for i in range(NCH):
                    acc_h = [
                        ps_acc.tile([128, 512], f32, tag="acc", name=f"acc_{i}_{h}")
                        for h in range(2)
                    ]
                    den = denp.tile([128, 1024], f16, tag="den")
                    nk = 4 * (i + 1)
                    for kk in range(nk):
                        ktile = kt[:, 128 * kk : 128 * (kk + 1)]
                        sc = ps_att.tile([128, 1024], f32, tag="att")
                        nc.tensor.matmul(
                            sc[:, 0:512],
                            ktile,
                            qt0[:, 512 * i : 512 * (i + 1)],
                            start=True,
                            stop=True,
                        )
                        nc.tensor.matmul(
                            sc[:, 512:1024],
                            ktile,
                            qt1[:, 512 * i : 512 * (i + 1)],
                            start=True,
                            stop=True,
                        )
                        pt = ptp.tile([128, 1024], bf16, tag="pt")
                        nc.scalar.activation(pt[:], sc[:], AF.Exp, scale=scale)
                        if kk >= 4 * i:
                            # causal per 512-half: keep iff q - p - 128*(kk-4i) >= 0
                            for hh in range(2):
                                nc.gpsimd.affine_select(
                                    pt[:, 512 * hh : 512 * (hh + 1)],
                                    pt[:, 512 * hh : 512 * (hh + 1)],
                                    [[1, 512]],
                                    ALU.is_ge,
                                    0.0,
                                    base=-128 * (kk - 4 * i),
                                    channel_multiplier=-1,
                                )
                        vtile = vnat[:, kk, :]
                        nc.tensor.matmul(
                            acc_h[0],
                            vtile,
                            pt[:, 0:512],
                            start=(kk == 0),
                            stop=(kk == nk - 1),
                            skip_group_check=True,
                        )
                        nc.tensor.matmul(
                            acc_h[1],
                            vtile,
                            pt[:, 512:1024],
                            start=(kk == 0),
                            stop=(kk == nk - 1),
                            skip_group_check=True,
                        )
                        if kk == 0:
                            nc.vector.tensor_copy(den[:], pt[:])
                        else:
                            nc.vector.tensor_add(den[:], den[:], pt[:])
                    atn_h = []
                    for h in range(2):
                        # reduce over keys AND broadcast across partitions in one MM
                        dps = ps_o.tile([128, 512], f32, tag="o", name=f"dps_{i}_{h}")
                        nc.tensor.matmul(
                            dps[:],
                            ones_m[:],
                            den[:, 512 * h : 512 * (h + 1)],
                            start=True,
                            stop=True,
                        )
                        rcb = rcpp.tile([128, 512], f32, tag="rc")
                        nc.vector.reciprocal(rcb[:], dps[:])
                        an = atnp.tile([128, 512], bf16, tag="atn")
                        nc.vector.tensor_mul(an[:], acc_h[h], rcb[:])
                        atn_h.append(an)

                    for t in range(4):
                        g = 4 * i + t
                        ost = ostp.tile([128, H], f16, tag="ost")
                        for nn in range(0, 4, 2):
                            poa = ps_o.tile([128, 512], f32, tag="o")
                            pob = ps_o.tile([128, 512], f32, tag="o")
                            for h in range(2):
                                lhs = atn_h[h][:, 128 * t : 128 * (t + 1)]
                                nc.tensor.matmul(
                                    poa[:],
                                    lhs,
                                    wo_sb[:, h, 512 * nn : 512 * (nn + 1)],
                                    start=(h == 0),
                                    stop=(h == 1),
                                    skip_group_check=True,
                                )
                                nc.tensor.matmul(
                                    pob[:],
                                    lhs,
                                    wo_sb[:, h, 512 * (nn + 1) : 512 * (nn + 2)],
                                    start=(h == 0),
                                    stop=(h == 1),
                                    skip_group_check=True,
                                )
                            # alternate drains between DVE and ACT to balance load
                            for ptile, col in ((poa, nn), (pob, nn + 1)):
                                if drain_flip % 4 != 0:
                                    nc.vector.tensor_copy(
                                        ost[:, 512 * col : 512 * (col + 1)], ptile[:]
                                    )
                                else:
                                    nc.scalar.copy(
                                        ost[:, 512 * col : 512 * (col + 1)], ptile[:]
                                    )
                                drain_flip += 1
                        nc.sync.dma_start(
                            out.ap()[128 * g : 128 * (g + 1), :], ost[:]
                        )

    nc.compile()
    return nc


_CACHE = {}


def _get_program(S=S_FULL):
    if S not in _CACHE:
        _CACHE[S] = build(S)
    return _CACHE[S]


def shard_inputs(hidden_states, wq, wk, wv, wo):
    import ml_dtypes

    bf = ml_dtypes.bfloat16
    S = hidden_states.shape[1]
    hid2d = np.ascontiguousarray(hidden_states.reshape(S, H).astype(bf))
    wqb = wq.astype(bf)
    wkb = wk.astype(bf)
    wvb = wv.astype(bf)
    wob = wo.astype(bf)
    in_maps = []
    for c in range(N_CORES):
        g = c // 2
        in_maps.append(
            {
                "hidden": hid2d,
                "wq_s": np.ascontiguousarray(wqb[:, 256 * c : 256 * (c + 1)]),
                "wk_s": np.ascontiguousarray(wkb[:, 128 * g : 128 * (g + 1)]),
                "wv_s": np.ascontiguousarray(wvb[:, 128 * g : 128 * (g + 1)]),
                "wo_s": np.ascontiguousarray(wob[256 * c : 256 * (c + 1), :]),
            }
        )
    return in_maps


def kernel(hidden_states, wq, wk, wv, wo, _trace=False):
    from concourse import bass_utils

    B, S, _ = hidden_states.shape
    nc = _get_program(S)
    in_maps = shard_inputs(hidden_states, wq, wk, wv, wo)
    res = bass_utils.run_bass_kernel_spmd(
        nc, in_maps, core_ids=list(range(N_CORES)), trace=_trace
    )
    acc = np.zeros((S, H), dtype=np.float32)
    for c in range(N_CORES):
        acc += res.results[c]["out_part"].astype(np.float32)
    out = acc.reshape(B, S, H)
    if _trace:
        return out, res
    return out


# revision 13
# speedup vs baseline: 1.8376x; 1.1642x over previous
"""Trainium2 Bass kernel for GQA multi-head attention (TP-8 over heads).

Problem: hidden [1, 4096, 2048] fp32; wq [2048, 2048], wk/wv [2048, 512],
wo [2048, 2048]; 16 q-heads / 4 kv-heads, head_dim 128, interleaved RoPE,
causal softmax attention, o_proj.

Sharding: core c in 0..7 handles q-heads {2c, 2c+1} and kv-head c//2
(kv proj duplicated across core pairs). Each core produces a partial
o_proj output [4096, 2048] (fp16); the host sums the 8 partials in fp32.

v3 structure (engine balance measured on HW):
- hidden + weights cast to bf16 on the host; hidden^T obtained on-device
  via DMA-transpose loads straight from the bf16 input (no roundtrip).
- QKV projection form A: stationary hT[k, s128], moving [wq0|wq1|wk|wv],
  natural-layout output.
- RoPE in natural layout, all-bf16, pair-swap via strided APs (DVE).
- Q/K transposed to [d, s] via PE-transpose + DVE drain; V kept natural.
- Attention in scores-transposed layout [k, q]: softmax needs no
  max-subtraction (logits are small), exp on ACT, causal mask via gpsimd
  affine_select, denominator accumulated in fp16 on DVE, normalization
  via ones-matmul reduce + gpsimd partition_broadcast of the reciprocal.
- o_proj: stationary attnT slices, PSUM drains split DVE/ACT.
"""

import sys

sys.path.insert(0, "/opt/trn_rl_repo")

import math

import numpy as np

NUM_HEADS = 16
NUM_KV = 4
HD = 128
H = 2048
KVD = 512
ROPE_BASE = 10000.0
S_FULL = 4096
N_CORES = 8


def _rope_tables(S):
    inv = 1.0 / (ROPE_BASE ** (np.arange(0, HD, 2, dtype=np.float64) / HD))
    t = np.arange(S, dtype=np.float64)
    fr = t[:, None] * inv[None, :]  # [S, 64]
    cos = np.repeat(np.cos(fr), 2, axis=1)
    sin = np.repeat(np.sin(fr), 2, axis=1)
    sin2 = sin.copy()
    sin2[:, 0::2] *= -1.0  # even dims get -sin (r1 = x1*cos - x2*sin)
    return cos, sin2


def _rope_tables_3x(S):
    import ml_dtypes

    cos, sin2 = _rope_tables(S)
    cos3 = np.tile(cos, (1, 3)).astype(ml_dtypes.bfloat16)
    sin3 = np.tile(sin2, (1, 3)).astype(ml_dtypes.bfloat16)
    return cos3, sin3


def build(S=S_FULL):
    import ml_dtypes

    import concourse.bacc as bacc
    import concourse.mybir as mybir
    import concourse.tile as tile

    f32 = mybir.dt.float32
    bf16 = mybir.dt.bfloat16
    f16 = mybir.dt.float16
    AF = mybir.ActivationFunctionType
    ALU = mybir.AluOpType

    NCH = S // 512
    NT = S // 128
    KT = H // 128
    scale = 1.0 / math.sqrt(HD)

    nc = bacc.Bacc("TRN2", target_bir_lowering=False, debug=False, num_devices=N_CORES)

    hid = nc.dram_tensor("hidden", [S, H], bf16, kind="ExternalInput")
    wq = nc.dram_tensor("wq_s", [H, 2 * HD], bf16, kind="ExternalInput")
    wk = nc.dram_tensor("wk_s", [H, HD], bf16, kind="ExternalInput")
    wv = nc.dram_tensor("wv_s", [H, HD], bf16, kind="ExternalInput")
    wo = nc.dram_tensor("wo_s", [2 * HD, H], bf16, kind="ExternalInput")
    out = nc.dram_tensor("out_part", [S, H], f16, kind="ExternalOutput")

    cos3_np, sin3_np = _rope_tables_3x(S)
    cos_d = nc.inline_tensor(cos3_np, name="cos_tab")
    sin_d = nc.inline_tensor(sin3_np, name="sin_tab")
    ident_d = nc.inline_tensor(np.eye(128, dtype=ml_dtypes.bfloat16), name="ident")

    with tile.TileContext(nc) as tc:
        with tc.tile_pool(name="pers", bufs=1) as pers:
            qt0 = pers.tile([128, S], bf16, tag="qt0")
            qt1 = pers.tile([128, S], bf16, tag="qt1")
            kt = pers.tile([128, S], bf16, tag="kt")
            vnat = pers.tile([128, NT, HD], bf16, tag="vnat")
            wcat = pers.tile([128, KT, 512], bf16, tag="wcat")
            wo_sb = pers.tile([128, 2, H], bf16, tag="wo")
            ones_m = pers.tile([128, 128], f16, tag="ones_m")
            ident_sb = pers.tile([128, 128], bf16, tag="ident")
            nc.vector.memset(ones_m[:], 1.0)
            nc.gpsimd.dma_start(ident_sb[:], ident_d.ap())

            nc.gpsimd.dma_start(
                wcat[:, :, 0 : 2 * HD],
                wq.ap().rearrange("(T p) m -> p T m", p=128),
            )
            nc.gpsimd.dma_start(
                wcat[:, :, 2 * HD : 3 * HD],
                wk.ap().rearrange("(T p) m -> p T m", p=128),
            )
            nc.gpsimd.dma_start(
                wcat[:, :, 3 * HD : 4 * HD],
                wv.ap().rearrange("(T p) m -> p T m", p=128),
            )
            nc.gpsimd.dma_start(
                wo_sb[:], wo.ap().rearrange("(T p) m -> p T m", p=128)
            )

            # ---- phase 1: transpose-load hidden, QKV proj, RoPE, Qt/Kt ----
            MC = 1024 if S % 1024 == 0 else 512  # macro-chunk rows per tr-load
            NTM = MC // 128
            with (
                tc.tile_pool(name="hT", bufs=2 * KT) as hTp,
                tc.tile_pool(name="trig", bufs=2) as trigp,
                tc.tile_pool(name="rope", bufs=6) as ropep,
                tc.tile_pool(name="psproj", bufs=2, space="PSUM") as ps_proj,
                tc.tile_pool(name="pstr", bufs=3, space="PSUM") as ps_tr,
            ):
                for i in range(S // MC):
                    cos_ch = trigp.tile([128, NTM, 3 * HD], bf16, tag="cos")
                    nc.gpsimd.dma_start(
                        cos_ch[:],
                        cos_d.ap()[MC * i : MC * (i + 1), :].rearrange(
                            "(q p) d -> p q d", p=128
                        ),
                    )
                    sin_ch = trigp.tile([128, NTM, 3 * HD], bf16, tag="sin")
                    nc.gpsimd.dma_start(
                        sin_ch[:],
                        sin_d.ap()[MC * i : MC * (i + 1), :].rearrange(
                            "(q p) d -> p q d", p=128
                        ),
                    )

                    hts = [
                        hTp.tile([128, MC], bf16, tag="hT", name=f"hT_{i}_{k}")
                        for k in range(KT)
                    ]
                    for k in range(KT):
                        # all transpose-DMAs on ONE queue: the xbar is shared
                        # hardware; concurrent transposes on two queues corrupt
                        nc.sync.dma_start_transpose(
                            hts[k][:],
                            hid.ap()[MC * i : MC * (i + 1), 128 * k : 128 * (k + 1)],
                        )

                    for t in range(NTM):
                        g = NTM * i + t
                        pq = ps_proj.tile([128, 512], f32, tag="proj")
                        for k in range(KT):
                            nc.tensor.matmul(
                                pq[:],
                                hts[k][:, 128 * t : 128 * (t + 1)],
                                wcat[:, k, :],
                                start=(k == 0),
                                stop=(k == KT - 1),
                            )
                        # drain whole tile to natural bf16 (includes V)
                        nat = ropep.tile([128, 512], bf16, tag="nat")
                        nc.scalar.copy(nat[:], pq[:])
                        nc.vector.tensor_copy(vnat[:, g, :], nat[:, 384:512])
                        # RoPE on q0|q1|k: rot = nat*cos + natswap*sin2
                        natp = nat[:, 0:384].rearrange("p (i two) -> p two i", two=2)
                        xsw = ropep.tile([128, 384], bf16, tag="xsw")
                        xwv = xsw[:].rearrange("p (i two) -> p two i", two=2)
                        nc.vector.tensor_copy(xwv[:, 0, :], natp[:, 1, :])
                        nc.vector.tensor_copy(xwv[:, 1, :], natp[:, 0, :])
                        t1 = ropep.tile([128, 384], bf16, tag="t1")
                        nc.vector.tensor_mul(t1[:], nat[:, 0:384], cos_ch[:, t, :])
                        rot = ropep.tile([128, 384], bf16, tag="rot")
                        nc.vector.tensor_mul(xsw[:], xsw[:], sin_ch[:, t, :])
                        nc.vector.tensor_add(rot[:], t1[:], xsw[:])
                        for j, dst in enumerate([qt0, qt1, kt]):
                            tp = ps_tr.tile([128, 128], bf16, tag="tr")
                            nc.tensor.transpose(
                                tp[:], rot[:, 128 * j : 128 * (j + 1)], ident_sb[:]
                            )
                            nc.vector.tensor_copy(
                                dst[:, 128 * g : 128 * (g + 1)], tp[:]
                            )

            # ---- phase 2: attention + o_proj (both heads per k-pass) ----
            with (
                tc.tile_pool(name="psatt", bufs=2, space="PSUM") as ps_att,
                tc.tile_pool(name="psacc", bufs=2, space="PSUM") as ps_acc,
                tc.tile_pool(name="pso", bufs=2, space="PSUM") as ps_o,
                tc.tile_pool(name="pt", bufs=4) as ptp,
                tc.tile_pool(name="den", bufs=2) as denp,
                tc.tile_pool(name="atn", bufs=4) as atnp,
                tc.tile_pool(name="rcp", bufs=2) as rcpp,
                tc.tile_pool(name="ost", bufs=2) as ostp,
            ):
                drain_flip = 0
                for i in range(NCH):
                    acc_h = [
                        ps_acc.tile([128, 512], f32, tag="acc", name=f"acc_{i}_{h}")
                        for h in range(2)
                    ]
                    den = denp.tile([128, 1024], f16, tag="den")
                    nk = 4 * (i + 1)
                    for kk in range(nk):
                        ktile = kt[:, 128 * kk : 128 * (kk + 1)]
                        sc = ps_att.tile([128, 1024], f32, tag="att")
                        nc.tensor.matmul(
                            sc[:, 0:512],
                            ktile,
                            qt0[:, 512 * i : 512 * (i + 1)],
                            start=True,
                            stop=True,
                        )
                        nc.tensor.matmul(
                            sc[:, 512:1024],
                            ktile,
                            qt1[:, 512 * i : 512 * (i + 1)],
                            start=True,
                            stop=True,
                        )
                        pt = ptp.tile([128, 1024], bf16, tag="pt")
                        nc.scalar.activation(pt[:], sc[:], AF.Exp, scale=scale)
                        if kk >= 4 * i:
                            # causal per 512-half: keep iff q - p - 128*(kk-4i) >= 0
                            for hh in range(2):
                                nc.gpsimd.affine_select(
                                    pt[:, 512 * hh : 512 * (hh + 1)],
                                    pt[:, 512 * hh : 512 * (hh + 1)],
                                    [[1, 512]],
                                    ALU.is_ge,
                                    0.0,
                                    base=-128 * (kk - 4 * i),
                                    channel_multiplier=-1,
                                )
                        vtile = vnat[:, kk, :]
                        nc.tensor.matmul(
                            acc_h[0],
                            vtile,
                            pt[:, 0:512],
                            start=(kk == 0),
                            stop=(kk == nk - 1),
                            skip_group_check=True,
                        )
                        nc.tensor.matmul(
                            acc_h[1],
                            vtile,
                            pt[:, 512:1024],
                            start=(kk == 0),
                            stop=(kk == nk - 1),
                            skip_group_check=True,
                        )
                        if kk == 0:
                            nc.vector.tensor_copy(den[:], pt[:])
                        else:
                            nc.vector.tensor_add(den[:], den[:], pt[:])
                    atn_h = []
                    for h in range(2):
                        # reduce over keys AND broadcast across partitions in one MM
                        dps = ps_o.tile([128, 512], f32, tag="o", name=f"dps_{i}_{h}")
                        nc.tensor.matmul(
                            dps[:],
                            ones_m[:],
                            den[:, 512 * h : 512 * (h + 1)],
                            start=True,
                            stop=True,
                        )
                        # drain acc to SBUF promptly (releases the PSUM bank for
                        # the next chunk), normalize in SBUF afterwards
                        au = rcpp.tile([128, 512], bf16, tag="au")
                        nc.scalar.copy(au[:], acc_h[h])
                        rcb = rcpp.tile([128, 512], f32, tag="rc")
                        nc.vector.reciprocal_approx_fast(rcb[:], dps[:])
                        an = atnp.tile([128, 512], bf16, tag="atn")
                        nc.vector.tensor_mul(an[:], au[:], rcb[:])
                        atn_h.append(an)

                    for t in range(4):
                        g = 4 * i + t
                        ost = ostp.tile([128, H], f16, tag="ost")
                        for nn in range(0, 4, 2):
                            poa = ps_o.tile([128, 512], f32, tag="o")
                            pob = ps_o.tile([128, 512], f32, tag="o")
                            for h in range(2):
                                lhs = atn_h[h][:, 128 * t : 128 * (t + 1)]
                                nc.tensor.matmul(
                                    poa[:],
                                    lhs,
                                    wo_sb[:, h, 512 * nn : 512 * (nn + 1)],
                                    start=(h == 0),
                                    stop=(h == 1),
                                    skip_group_check=True,
                                )
                                nc.tensor.matmul(
                                    pob[:],
                                    lhs,
                                    wo_sb[:, h, 512 * (nn + 1) : 512 * (nn + 2)],
                                    start=(h == 0),
                                    stop=(h == 1),
                                    skip_group_check=True,
                                )
                            # alternate drains between DVE and ACT to balance load
                            for ptile, col in ((poa, nn), (pob, nn + 1)):
                                if drain_flip % 4 != 0:
                                    nc.vector.tensor_copy(
                                        ost[:, 512 * col : 512 * (col + 1)], ptile[:]
                                    )
                                else:
                                    nc.scalar.copy(
                                        ost[:, 512 * col : 512 * (col + 1)], ptile[:]
                                    )
                                drain_flip += 1
                        nc.sync.dma_start(
                            out.ap()[128 * g : 128 * (g + 1), :], ost[:]
                        )

    nc.compile()
    return nc


_CACHE = {}


def _get_program(S=S_FULL):
    if S not in _CACHE:
        _CACHE[S] = build(S)
    return _CACHE[S]


def shard_inputs(hidden_states, wq, wk, wv, wo):
    import ml_dtypes

    bf = ml_dtypes.bfloat16
    S = hidden_states.shape[1]
    hid2d = np.ascontiguousarray(hidden_states.reshape(S, H).astype(bf))
    wqb = wq.astype(bf)
    wkb = wk.astype(bf)
    wvb = wv.astype(bf)
    wob = wo.astype(bf)
    in_maps = []
    for c in range(N_CORES):
        g = c // 2
        in_maps.append(
            {
                "hidden": hid2d,
                "wq_s": np.ascontiguousarray(wqb[:, 256 * c : 256 * (c + 1)]),
                "wk_s": np.ascontiguousarray(wkb[:, 128 * g : 128 * (g + 1)]),
                "wv_s": np.ascontiguousarray(wvb[:, 128 * g : 128 * (g + 1)]),
                "wo_s": np.ascontiguousarray(wob[256 * c : 256 * (c + 1), :]),
            }
        )
    return in_maps


def kernel(hidden_states, wq, wk, wv, wo, _trace=False):
    from concourse import bass_utils

    B, S, _ = hidden_states.shape
    nc = _get_program(S)
    in_maps = shard_inputs(hidden_states, wq, wk, wv, wo)
    res = bass_utils.run_bass_kernel_spmd(
        nc, in_maps, core_ids=list(range(N_CORES)), trace=_trace
    )
    acc = np.zeros((S, H), dtype=np.float32)
    for c in range(N_CORES):
        acc += res.results[c]["out_part"].astype(np.float32)
    out = acc.reshape(B, S, H)
    if _trace:
        return out, res
    return out
